# revision 10
# baseline (speedup 1.0000x reference)
"""CrossAttention kernel for 8 Trainium2 NeuronCores (Bass/Tile).

Problem (hardcoded): x [4,2048,1024] f32, context [4,2048,1024] f32,
mask [4,2048] bool, Wq/Wk/Wv [1024,512], Wo [512,1024], bo [1024].
8 heads x 64 dim, scale 1/8, out = softmax(q k^T * s + maskbias) v @ Wo + bo.

Sharding: core c -> (batch b = c//2, head-group hg = c%2 of 4 heads).
Each core computes a partial output [2048,1024] (its 4 heads through its
256-row slice of Wo); the host sums core pairs and adds bo.

Device-side layout trick: everything is computed in "transposed" form so
no on-device transposes are needed:
  qT/kT = W^T @ x^T come out of the projection matmul as [d, rows].
  sim is computed as simT [j, i]  (lhsT=kT tile, rhs=qT tile), so the
  context mask/padding bias is per-partition -> fused into the ACT exp
  (exp(sim*scale + bias)) together with the attention scale.
  PV uses expT directly as the moving operand with v' = [v | ones] as the
  stationary one; the ones column yields the softmax denominator for free.
  The PV output [d, i] is exactly the lhsT the Wo projection needs.

The context rows where mask=False are removed on the host (their softmax
weight is exactly zero), and the remainder padded to a multiple of 128
with bias -1e30 rows.
"""

import math

import numpy as np
import ml_dtypes

BF16 = ml_dtypes.bfloat16

B, N, DIM = 4, 2048, 1024
HEADS, DH = 8, 64
INNER = HEADS * DH  # 512
HG = INNER // 2  # 256 per head-group
IC = 1024  # i-chunk for attention inner loop

_PROGRAMS: dict[tuple, object] = {}


def _build_program(m_pad: int, repeats: int = 1):
    import concourse.tile as tile
    from concourse import bacc, mybir

    f32 = mybir.dt.float32
    bf16 = mybir.dt.bfloat16
    Exp = mybir.ActivationFunctionType.Exp
    mpt = m_pad // 128

    nc = bacc.Bacc("TRN2", target_bir_lowering=False, debug=False)
    xT_d = nc.dram_tensor("xT", [DIM, N], bf16, kind="ExternalInput").ap()
    cT_d = nc.dram_tensor("ctxT", [DIM, m_pad], bf16, kind="ExternalInput").ap()
    wq_d = nc.dram_tensor("wq", [DIM, HG], bf16, kind="ExternalInput").ap()
    wk_d = nc.dram_tensor("wk", [DIM, HG], bf16, kind="ExternalInput").ap()
    wv_d = nc.dram_tensor("wv", [DIM, HG], bf16, kind="ExternalInput").ap()
    wo_d = nc.dram_tensor("wo", [HG, DIM], bf16, kind="ExternalInput").ap()
    bias_d = nc.dram_tensor("bias", [128, mpt], f32, kind="ExternalInput").ap()
    out_d = nc.dram_tensor("out", [N, DIM], f32, kind="ExternalOutput").ap()

    with tile.TileContext(nc) as tc:
        with tc.tile_pool(name="const", bufs=1) as const, tc.tile_pool(
            name="work", bufs=4
        ) as work, tc.tile_pool(name="outp", bufs=3) as outp:
            xT = const.tile([128, 8, N], bf16)
            cT = const.tile([128, 8, m_pad], bf16)
            wq = const.tile([128, 8, HG], bf16)
            wk = const.tile([128, 8, HG], bf16)
            wv = const.tile([128, 8, HG], bf16)
            wo = const.tile([128, 2, DIM], bf16)
            biasv = const.tile([128, mpt], f32)
            qT = const.tile([128, 2, N], bf16)
            kT = const.tile([128, 2, m_pad], bf16)
            vp = const.tile([128, mpt, 4 * (DH + 1)], bf16)
            oT = const.tile([128, 2, N], bf16)

            for kt in range(8):
                s = slice(kt * 128, (kt + 1) * 128)
                nc.sync.dma_start(out=xT[:, kt, :], in_=xT_d[s, :])
                nc.sync.dma_start(out=cT[:, kt, :], in_=cT_d[s, :])
                nc.sync.dma_start(out=wq[:, kt, :], in_=wq_d[s, :])
                nc.sync.dma_start(out=wk[:, kt, :], in_=wk_d[s, :])
                nc.sync.dma_start(out=wv[:, kt, :], in_=wv_d[s, :])
            nc.sync.dma_start(out=wo[:, 0, :], in_=wo_d[0:128, :])
            nc.sync.dma_start(out=wo[:, 1, :], in_=wo_d[128:256, :])
            nc.sync.dma_start(out=biasv[:, :], in_=bias_d[:, :])
            nc.vector.memset(vp[:, :, :], 1.0)

            def emit_body():
                # ---- phase 1: q/k/v projections ---------------------------
                with tc.tile_pool(
                    name="pj", bufs=2, space="PSUM"
                ) as pjp, tc.tile_pool(name="pvv", bufs=2, space="PSUM") as pvp:
                    for m2 in range(2):
                        ws = slice(m2 * 128, (m2 + 1) * 128)
                        for icc in range(N // 512):
                            cs = slice(icc * 512, (icc + 1) * 512)
                            ps = pjp.tile([128, 512], f32, tag="pq")
                            for kt in range(8):
                                nc.tensor.matmul(
                                    ps[:, :],
                                    lhsT=wq[:, kt, ws],
                                    rhs=xT[:, kt, cs],
                                    start=(kt == 0),
                                    stop=(kt == 7),
                                )
                            nc.vector.tensor_copy(out=qT[:, m2, cs], in_=ps[:, :])
                        j0 = 0
                        while j0 < m_pad:
                            jl = min(512, m_pad - j0)
                            ps = pjp.tile([128, 512], f32, tag="pq")
                            for kt in range(8):
                                nc.tensor.matmul(
                                    ps[:, :jl],
                                    lhsT=wk[:, kt, ws],
                                    rhs=cT[:, kt, j0 : j0 + jl],
                                    start=(kt == 0),
                                    stop=(kt == 7),
                                )
                            nc.vector.tensor_copy(
                                out=kT[:, m2, j0 : j0 + jl], in_=ps[:, :jl]
                            )
                            j0 += jl
                    for jt in range(mpt):
                        js = slice(jt * 128, (jt + 1) * 128)
                        ps = pvp.tile([128, HG], f32, tag="pv")
                        for kt in range(8):
                            nc.tensor.matmul(
                                ps[:, :],
                                lhsT=cT[:, kt, js],
                                rhs=wv[:, kt, :],
                                start=(kt == 0),
                                stop=(kt == 7),
                            )
                        for lh in range(4):
                            nc.vector.tensor_copy(
                                out=vp[:, jt, lh * 65 : lh * 65 + 64],
                                in_=ps[:, lh * 64 : (lh + 1) * 64],
                            )

                # ---- phase 2: attention ----------------------------------
                # Heads are processed in pairs (2p, 2p+1). The two K=64 sim
                # matmuls land in PE row-groups 0-1 / 2-3 (auto tile_position
                # from lhsT base partition 0/64) and run concurrently,
                # writing the two 512-halves (= 2 banks) of one psum tile;
                # one ACT exp covers both heads.
                with tc.tile_pool(
                    name="sm", bufs=2, space="PSUM"
                ) as simp, tc.tile_pool(name="ac", bufs=2, space="PSUM") as accp:
                    for pr in range(2):
                        va = slice((2 * pr) * 65, (2 * pr + 1) * 65)
                        vb = slice((2 * pr + 1) * 65, (2 * pr + 2) * 65)
                        for ic in range(N // 512):
                            i0 = ic * 512
                            qs = slice(i0, i0 + 512)
                            acc0 = accp.tile([65, 512], f32, tag="acc0")
                            acc1 = accp.tile([65, 512], f32, tag="acc1")
                            for jt in range(mpt):
                                js = slice(jt * 128, (jt + 1) * 128)
                                sim = simp.tile([128, 1024], f32, tag="sim")
                                nc.tensor.matmul(
                                    sim[:, 0:512],
                                    lhsT=kT[0:64, pr, js],
                                    rhs=qT[0:64, pr, qs],
                                    start=True,
                                    stop=True,
                                )
                                nc.tensor.matmul(
                                    sim[:, 512:1024],
                                    lhsT=kT[64:128, pr, js],
                                    rhs=qT[64:128, pr, qs],
                                    start=True,
                                    stop=True,
                                )
                                ex = work.tile([128, 1024], bf16, tag="exp")
                                nc.scalar.activation(
                                    out=ex[:, :],
                                    in_=sim[:, :],
                                    func=Exp,
                                    bias=biasv[:, jt : jt + 1],
                                    scale=0.125,
                                )
                                nc.tensor.matmul(
                                    acc0[:, :],
                                    lhsT=vp[:, jt, va],
                                    rhs=ex[:, 0:512],
                                    start=(jt == 0),
                                    stop=(jt == mpt - 1),
                                )
                                nc.tensor.matmul(
                                    acc1[:, :],
                                    lhsT=vp[:, jt, vb],
                                    rhs=ex[:, 512:1024],
                                    start=(jt == 0),
                                    stop=(jt == mpt - 1),
                                )
                            # normalize: oT = acc[0:64] * (1/acc[64]) bcast
                            for hh, acc in ((0, acc0), (1, acc1)):
                                rc = work.tile([1, 512], f32, tag="recip")
                                nc.vector.reciprocal(out=rc[:, :], in_=acc[64:65, :])
                                bc = work.tile([64, 512], f32, tag="bcast")
                                nc.gpsimd.partition_broadcast(bc[:, :], rc[:, :])
                                if hh == 0:
                                    nc.vector.tensor_mul(
                                        oT[0:64, pr, qs], acc[0:64, :], bc[:, :]
                                    )
                                else:
                                    st = work.tile([64, 512], bf16, tag="stage")
                                    nc.vector.tensor_mul(
                                        st[:, :], acc[0:64, :], bc[:, :]
                                    )
                                    nc.sync.dma_start(
                                        out=oT[64:128, pr, qs], in_=st[:, :]
                                    )

                # ---- phase 3: output projection --------------------------
                with tc.tile_pool(name="po", bufs=2, space="PSUM") as pop:
                    for it in range(N // 128):
                        ts_ = slice(it * 128, (it + 1) * 128)
                        po = pop.tile([128, DIM], f32, tag="po")
                        for nh2 in range(2):
                            ns = slice(nh2 * 512, (nh2 + 1) * 512)
                            for ck2 in range(2):
                                nc.tensor.matmul(
                                    po[:, ns],
                                    lhsT=oT[:, ck2, ts_],
                                    rhs=wo[:, ck2, ns],
                                    start=(ck2 == 0),
                                    stop=(ck2 == 1),
                                )
                        ob = outp.tile([128, DIM], f32, tag="ob")
                        nc.vector.tensor_copy(out=ob[:, :], in_=po[:, :])
                        nc.sync.dma_start(out=out_d[ts_, :], in_=ob[:, :])

            for _ in range(repeats):
                emit_body()

    nc.compile()
    return nc


def _get_program(m_pad: int, repeats: int = 1):
    key = (m_pad, repeats)
    if key not in _PROGRAMS:
        _PROGRAMS[key] = _build_program(m_pad, repeats)
    return _PROGRAMS[key]


def make_in_maps(x, context, mask, Wq, Wk, Wv, Wo):
    """Host-side sharding: returns (m_pad, list of 8 per-core input dicts)."""
    x = np.asarray(x, dtype=np.float32)
    context = np.asarray(context, dtype=np.float32)
    mask = np.asarray(mask)
    idxs = []
    for b in range(B):
        idx = np.nonzero(mask[b])[0]
        if idx.size == 0:
            # all masked -> reference softmax degenerates to uniform over all
            idx = np.arange(context.shape[1])
        idxs.append(idx)
    m_pad = max(128, 128 * math.ceil(max(i.size for i in idxs) / 128))

    wq8 = np.asarray(Wq, dtype=np.float32)
    wk8 = np.asarray(Wk, dtype=np.float32)
    wv8 = np.asarray(Wv, dtype=np.float32)
    wo8 = np.asarray(Wo, dtype=np.float32)

    in_maps = []
    for c in range(8):
        b, hg = c // 2, c % 2
        idx = idxs[b]
        mb = idx.size
        xT = np.ascontiguousarray(x[b].T).astype(BF16)
        cTt = np.zeros((DIM, m_pad), dtype=BF16)
        cTt[:, :mb] = np.ascontiguousarray(context[b][idx].T)
        biasv = np.full((m_pad,), -1e30, dtype=np.float32)
        biasv[:mb] = 0.0
        bias_t = np.ascontiguousarray(biasv.reshape(m_pad // 128, 128).T)
        s = slice(hg * HG, (hg + 1) * HG)
        in_maps.append(
            {
                "xT": xT,
                "ctxT": cTt,
                "bias": bias_t,
                "wq": wq8[:, s].astype(BF16),
                "wk": wk8[:, s].astype(BF16),
                "wv": wv8[:, s].astype(BF16),
                "wo": np.ascontiguousarray(wo8[s, :]).astype(BF16),
            }
        )
    return m_pad, in_maps


def kernel(x, context, mask, Wq, Wk, Wv, Wo, bo):
    from concourse.bass_utils import run_bass_kernel_spmd

    m_pad, in_maps = make_in_maps(x, context, mask, Wq, Wk, Wv, Wo)
    nc = _get_program(m_pad)
    res = run_bass_kernel_spmd(nc, in_maps, core_ids=list(range(8))).results
    out = np.empty((B, N, DIM), dtype=np.float32)
    bo32 = np.asarray(bo, dtype=np.float32)
    for b in range(B):
        out[b] = res[2 * b]["out"] + res[2 * b + 1]["out"] + bo32
    return out


# revision 11
# speedup vs baseline: 1.0165x; 1.0165x over previous
"""CrossAttention kernel for 8 Trainium2 NeuronCores (Bass/Tile).

Problem (hardcoded): x [4,2048,1024] f32, context [4,2048,1024] f32,
mask [4,2048] bool, Wq/Wk/Wv [1024,512], Wo [512,1024], bo [1024].
8 heads x 64 dim, scale 1/8, out = softmax(q k^T * s + maskbias) v @ Wo + bo.

Sharding: core c -> (batch b = c//2, head-group hg = c%2 of 4 heads).
Each core computes a partial output [2048,1024] (its 4 heads through its
256-row slice of Wo); the host sums core pairs and adds bo.

Device-side layout trick: everything is computed in "transposed" form so
no on-device transposes are needed:
  qT/kT = W^T @ x^T come out of the projection matmul as [d, rows].
  sim is computed as simT [j, i]  (lhsT=kT tile, rhs=qT tile), so the
  context mask/padding bias is per-partition -> fused into the ACT exp
  (exp(sim*scale + bias)) together with the attention scale.
  PV uses expT directly as the moving operand with v' = [v | ones] as the
  stationary one; the ones column yields the softmax denominator for free.
  The PV output [d, i] is exactly the lhsT the Wo projection needs.

The context rows where mask=False are removed on the host (their softmax
weight is exactly zero), and the remainder padded to a multiple of 128
with bias -1e30 rows.
"""

import math

import numpy as np
import ml_dtypes

BF16 = ml_dtypes.bfloat16

B, N, DIM = 4, 2048, 1024
HEADS, DH = 8, 64
INNER = HEADS * DH  # 512
HG = INNER // 2  # 256 per head-group
IC = 1024  # i-chunk for attention inner loop

_PROGRAMS: dict[tuple, object] = {}


def _build_program(m_pad: int, repeats: int = 1):
    import concourse.tile as tile
    from concourse import bacc, mybir

    f32 = mybir.dt.float32
    bf16 = mybir.dt.bfloat16
    Exp = mybir.ActivationFunctionType.Exp
    mpt = m_pad // 128

    nc = bacc.Bacc("TRN2", target_bir_lowering=False, debug=False)
    xT_d = nc.dram_tensor("xT", [DIM, N], bf16, kind="ExternalInput").ap()
    cT_d = nc.dram_tensor("ctxT", [DIM, m_pad], bf16, kind="ExternalInput").ap()
    wq_d = nc.dram_tensor("wq", [DIM, HG], bf16, kind="ExternalInput").ap()
    wk_d = nc.dram_tensor("wk", [DIM, HG], bf16, kind="ExternalInput").ap()
    wv_d = nc.dram_tensor("wv", [DIM, HG], bf16, kind="ExternalInput").ap()
    wo_d = nc.dram_tensor("wo", [HG, DIM], bf16, kind="ExternalInput").ap()
    bias_d = nc.dram_tensor("bias", [128, mpt], f32, kind="ExternalInput").ap()
    out_d = nc.dram_tensor("out", [N, DIM], f32, kind="ExternalOutput").ap()

    with tile.TileContext(nc) as tc:
        with tc.tile_pool(name="const", bufs=1) as const, tc.tile_pool(
            name="work", bufs=4
        ) as work, tc.tile_pool(name="outp", bufs=3) as outp:
            xT = const.tile([128, 8, N], bf16)
            cT = const.tile([128, 8, m_pad], bf16)
            wq = const.tile([128, 8, HG], bf16)
            wk = const.tile([128, 8, HG], bf16)
            wv = const.tile([128, 8, HG], bf16)
            wo = const.tile([128, 2, DIM], bf16)
            biasv = const.tile([128, mpt], f32)
            qT = const.tile([128, 2, N], bf16)
            kT = const.tile([128, 2, m_pad], bf16)
            vp = const.tile([128, mpt, 4 * (DH + 1)], bf16)
            oT = const.tile([128, 2, N], bf16)

            for kt in range(8):
                s = slice(kt * 128, (kt + 1) * 128)
                nc.sync.dma_start(out=xT[:, kt, :], in_=xT_d[s, :])
                nc.sync.dma_start(out=cT[:, kt, :], in_=cT_d[s, :])
                nc.sync.dma_start(out=wq[:, kt, :], in_=wq_d[s, :])
                nc.sync.dma_start(out=wk[:, kt, :], in_=wk_d[s, :])
                nc.sync.dma_start(out=wv[:, kt, :], in_=wv_d[s, :])
            nc.sync.dma_start(out=wo[:, 0, :], in_=wo_d[0:128, :])
            nc.sync.dma_start(out=wo[:, 1, :], in_=wo_d[128:256, :])
            nc.sync.dma_start(out=biasv[:, :], in_=bias_d[:, :])
            nc.vector.memset(vp[:, :, :], 1.0)

            def emit_body(psp):
                # One persistent psum pool for the whole body so phases
                # overlap on pure dataflow deps (no pool-boundary WAR
                # barriers). Budget: tag "sim" [128,1024] x2 bufs = 4 banks,
                # tags "acc0"/"acc1" x2 bufs = 4 banks -> 8 exactly.

                # ---- v projection (needed first by attention PV) ----------
                for jt in range(mpt):
                    js = slice(jt * 128, (jt + 1) * 128)
                    ps = psp.tile([128, HG], f32, tag="acc" + str(jt % 2))
                    for kt in range(8):
                        nc.tensor.matmul(
                            ps[:, :],
                            lhsT=cT[:, kt, js],
                            rhs=wv[:, kt, :],
                            start=(kt == 0),
                            stop=(kt == 7),
                        )
                    for lh in range(4):
                        nc.vector.tensor_copy(
                            out=vp[:, jt, lh * 65 : lh * 65 + 64],
                            in_=ps[:, lh * 64 : (lh + 1) * 64],
                        )

                for pr in range(2):
                    # ---- k/q projections for this head pair ---------------
                    ws = slice(pr * 128, (pr + 1) * 128)
                    j0 = 0
                    while j0 < m_pad:
                        jl = min(512, m_pad - j0)
                        ps = psp.tile([128, 512], f32, tag="sim")
                        for kt in range(8):
                            nc.tensor.matmul(
                                ps[:, :jl],
                                lhsT=wk[:, kt, ws],
                                rhs=cT[:, kt, j0 : j0 + jl],
                                start=(kt == 0),
                                stop=(kt == 7),
                            )
                        nc.vector.tensor_copy(
                            out=kT[:, pr, j0 : j0 + jl], in_=ps[:, :jl]
                        )
                        j0 += jl
                    for icc in range(N // 512):
                        cs = slice(icc * 512, (icc + 1) * 512)
                        ps = psp.tile([128, 512], f32, tag="sim")
                        for kt in range(8):
                            nc.tensor.matmul(
                                ps[:, :],
                                lhsT=wq[:, kt, ws],
                                rhs=xT[:, kt, cs],
                                start=(kt == 0),
                                stop=(kt == 7),
                            )
                        nc.vector.tensor_copy(out=qT[:, pr, cs], in_=ps[:, :])

                    # ---- attention for head pair (2pr, 2pr+1) -------------
                    # The two K=64 sim matmuls land in PE row-groups 0-1 /
                    # 2-3 (auto tile_position from lhsT base partition 0/64)
                    # and run concurrently, writing the two 512-halves (= 2
                    # banks) of one psum tile; one ACT exp covers both heads.
                    va = slice((2 * pr) * 65, (2 * pr + 1) * 65)
                    vb = slice((2 * pr + 1) * 65, (2 * pr + 2) * 65)
                    for ic in range(N // 512):
                        i0 = ic * 512
                        qs = slice(i0, i0 + 512)
                        acc0 = psp.tile([65, 512], f32, tag="acc0")
                        acc1 = psp.tile([65, 512], f32, tag="acc1")
                        for jt in range(mpt):
                            js = slice(jt * 128, (jt + 1) * 128)
                            sim = psp.tile([128, 1024], f32, tag="sim")
                            nc.tensor.matmul(
                                sim[:, 0:512],
                                lhsT=kT[0:64, pr, js],
                                rhs=qT[0:64, pr, qs],
                                start=True,
                                stop=True,
                            )
                            nc.tensor.matmul(
                                sim[:, 512:1024],
                                lhsT=kT[64:128, pr, js],
                                rhs=qT[64:128, pr, qs],
                                start=True,
                                stop=True,
                            )
                            ex = work.tile([128, 1024], bf16, tag="exp")
                            nc.scalar.activation(
                                out=ex[:, :],
                                in_=sim[:, :],
                                func=Exp,
                                bias=biasv[:, jt : jt + 1],
                                scale=0.125,
                            )
                            nc.tensor.matmul(
                                acc0[:, :],
                                lhsT=vp[:, jt, va],
                                rhs=ex[:, 0:512],
                                start=(jt == 0),
                                stop=(jt == mpt - 1),
                            )
                            nc.tensor.matmul(
                                acc1[:, :],
                                lhsT=vp[:, jt, vb],
                                rhs=ex[:, 512:1024],
                                start=(jt == 0),
                                stop=(jt == mpt - 1),
                            )
                        # normalize: oT = acc[0:64] * (1/acc[64]) bcast
                        for hh, acc in ((0, acc0), (1, acc1)):
                            rc = work.tile([1, 512], f32, tag="recip")
                            nc.vector.reciprocal(out=rc[:, :], in_=acc[64:65, :])
                            bc = work.tile([64, 512], f32, tag="bcast")
                            nc.gpsimd.partition_broadcast(bc[:, :], rc[:, :])
                            if hh == 0:
                                nc.vector.tensor_mul(
                                    oT[0:64, pr, qs], acc[0:64, :], bc[:, :]
                                )
                            else:
                                st = work.tile([64, 512], bf16, tag="stage")
                                nc.vector.tensor_mul(st[:, :], acc[0:64, :], bc[:, :])
                                nc.sync.dma_start(
                                    out=oT[64:128, pr, qs], in_=st[:, :]
                                )

                # ---- output projection -----------------------------------
                for it in range(N // 128):
                    ts_ = slice(it * 128, (it + 1) * 128)
                    po = psp.tile([128, DIM], f32, tag="sim")
                    for nh2 in range(2):
                        ns = slice(nh2 * 512, (nh2 + 1) * 512)
                        for ck2 in range(2):
                            nc.tensor.matmul(
                                po[:, ns],
                                lhsT=oT[:, ck2, ts_],
                                rhs=wo[:, ck2, ns],
                                start=(ck2 == 0),
                                stop=(ck2 == 1),
                            )
                    ob = outp.tile([128, DIM], f32, tag="ob")
                    nc.vector.tensor_copy(out=ob[:, :], in_=po[:, :])
                    nc.sync.dma_start(out=out_d[ts_, :], in_=ob[:, :])

            with tc.tile_pool(name="ps", bufs=2, space="PSUM") as psp:
                for _ in range(repeats):
                    emit_body(psp)

    nc.compile()
    return nc


def _get_program(m_pad: int, repeats: int = 1):
    key = (m_pad, repeats)
    if key not in _PROGRAMS:
        _PROGRAMS[key] = _build_program(m_pad, repeats)
    return _PROGRAMS[key]


def make_in_maps(x, context, mask, Wq, Wk, Wv, Wo):
    """Host-side sharding: returns (m_pad, list of 8 per-core input dicts)."""
    x = np.asarray(x, dtype=np.float32)
    context = np.asarray(context, dtype=np.float32)
    mask = np.asarray(mask)
    idxs = []
    for b in range(B):
        idx = np.nonzero(mask[b])[0]
        if idx.size == 0:
            # all masked -> reference softmax degenerates to uniform over all
            idx = np.arange(context.shape[1])
        idxs.append(idx)
    m_pad = max(128, 128 * math.ceil(max(i.size for i in idxs) / 128))

    wq8 = np.asarray(Wq, dtype=np.float32)
    wk8 = np.asarray(Wk, dtype=np.float32)
    wv8 = np.asarray(Wv, dtype=np.float32)
    wo8 = np.asarray(Wo, dtype=np.float32)

    in_maps = []
    for c in range(8):
        b, hg = c // 2, c % 2
        idx = idxs[b]
        mb = idx.size
        xT = np.ascontiguousarray(x[b].T).astype(BF16)
        cTt = np.zeros((DIM, m_pad), dtype=BF16)
        cTt[:, :mb] = np.ascontiguousarray(context[b][idx].T)
        biasv = np.full((m_pad,), -1e30, dtype=np.float32)
        biasv[:mb] = 0.0
        bias_t = np.ascontiguousarray(biasv.reshape(m_pad // 128, 128).T)
        s = slice(hg * HG, (hg + 1) * HG)
        in_maps.append(
            {
                "xT": xT,
                "ctxT": cTt,
                "bias": bias_t,
                "wq": wq8[:, s].astype(BF16),
                "wk": wk8[:, s].astype(BF16),
                "wv": wv8[:, s].astype(BF16),
                "wo": np.ascontiguousarray(wo8[s, :]).astype(BF16),
            }
        )
    return m_pad, in_maps


def kernel(x, context, mask, Wq, Wk, Wv, Wo, bo):
    from concourse.bass_utils import run_bass_kernel_spmd

    m_pad, in_maps = make_in_maps(x, context, mask, Wq, Wk, Wv, Wo)
    nc = _get_program(m_pad)
    res = run_bass_kernel_spmd(nc, in_maps, core_ids=list(range(8))).results
    out = np.empty((B, N, DIM), dtype=np.float32)
    bo32 = np.asarray(bo, dtype=np.float32)
    for b in range(B):
        out[b] = res[2 * b]["out"] + res[2 * b + 1]["out"] + bo32
    return out
